# revision 1
# baseline (speedup 1.0000x reference)
"""Trainium2 Bass kernel for nn_CrossAttention_DenseAVInteractions.

Math: the reference builds a cartesian KV grid kv[b,i,j] = pv[b,i] + pa[b,j]
over (N_v, N_a) and attends 64 queries against all N_v*N_a = 65536 keys.
Because the logits decompose as s[q,(i,j)] = (q.k_v[i]) + (q.k_a[j]), the
softmax over the product grid factorizes exactly:

    p[q,(i,j)] = softmax_i(q.k_v)[q,i] * softmax_j(q.k_a)[q,j]
    out[q]     = softmax_i(q.k_v) @ v_v + softmax_j(q.k_a) @ v_a

so the whole attention reduces to two 256-key attentions per (b, h).

Sharding (8 cores): core c handles batch b = c // 4 and the head pair
(2j, 2j+1) with j = c % 4.  Each core computes its heads' partial output
projection partial = out_heads @ Wproj[:, head_cols].T in f32; the host sums
the 4 partials per batch and adds bproj.

Device-side design (v3 — informed by two profile rounds):
 - Everything is bf16 on the wire and in the PE (measured end-to-end rel-err
   ~5e-3, well under the 2e-2 gate): one [128, 5376] bf16 packed input per
   core, streamed as 8 DMAs over the two HWDGE queues in consumption order.
   Chunks are k-tile-interleaved so each projection can start as soon as its
   first k-tiles land.
 - V is projected directly into [token, channel] layout (tokens on
   partitions) so no PE transpose / extra PSUM round-trip for V.
 - Softmax normalization is folded into the P transpose: transpose p against
   diag(1/z) instead of the identity (exp's accum_out provides z for free).
   The identity is built on-device (memset + affine_select) - no DMA.
 - The PE warmup (~24 small bf16 matmuls on junk) spans the whole DMA wait
   so the HAM clock gate is 8/8 when real matmuls start (cold PE runs at
   1.2 GHz vs 2.4 GHz warm).
 - The score->exp->1/z->transpose chains are emitted under
   tc.high_priority() so the Tile scheduler doesn't push them behind bulk
   projection matmuls (observed in round 2: +2us on the critical path).
"""

import os
import sys

import numpy as np

sys.path.insert(0, "/opt/trn_rl_repo")

import ml_dtypes

BF16 = ml_dtypes.bfloat16

DIM = 512
H = 8
HD = DIM // H          # 64
B = 2
N_MM = 64
N_A = 256
N_V = 256
SCALE = HD ** -0.5     # 0.125
N_CORES = 8

PACK_COLS = 5376

# (lo, hi, engine) in emission order; per-engine order = HW queue FIFO order.
CHUNKS = [
    (0, 768, "sync"),        # wkv k01 + xv k01   (k_v projection: deep chain)
    (2560, 3328, "scalar"),  # wq + xmm           (q: needed by both scores)
    (768, 1536, "sync"),     # wkv k23 + xv k23
    (3328, 4096, "scalar"),  # wka k01 + xa k01   (k_a: second deep chain)
    (4096, 4864, "scalar"),  # wka k23 + xa k23
    (1536, 2048, "sync"),    # wvv                (v_v projection)
    (2048, 2560, "sync"),    # wva                (v_a projection)
    (4864, 5376, "scalar"),  # wproj              (final projection: no tail)
]

# (tensor, k-tile) -> (abs col offset, width) in the packed layout above.
def _seg_off(name, k):
    if name == "wkv":
        return 768 * (k // 2) + 128 * (k % 2), 128
    if name == "xv":
        return 768 * (k // 2) + 256 + 256 * (k % 2), 256
    if name == "wvv":
        return 1536 + 128 * k, 128
    if name == "wva":
        return 2048 + 128 * k, 128
    if name == "wq":
        return 2560 + 128 * k, 128
    if name == "xmm":
        return 3072 + 64 * k, 64
    if name == "wka":
        return 3328 + 768 * (k // 2) + 128 * (k % 2), 128
    if name == "xa":
        return 3328 + 768 * (k // 2) + 256 + 256 * (k % 2), 256
    if name == "wproj":
        return 4864, 512
    raise KeyError(name)


_cached = {}


def _build_program():
    import concourse.bacc as bacc
    from concourse import mybir
    from concourse.tile import TileContext

    f32 = mybir.dt.float32
    bf16 = mybir.dt.bfloat16
    nc = bacc.Bacc(name="cross_attn_dense_av")

    packA = nc.dram_tensor("packA", [128, PACK_COLS], bf16, kind="ExternalInput")
    out_d = nc.dram_tensor("out", [64, 512], f32, kind="ExternalOutput")

    from contextlib import ExitStack

    with TileContext(nc) as tc, ExitStack() as ctx:
        io = ctx.enter_context(tc.tile_pool(name="io", bufs=1))
        work = ctx.enter_context(tc.tile_pool(name="work", bufs=1))
        ps_mm = ctx.enter_context(tc.tile_pool(name="ps_mm", bufs=3, space="PSUM"))
        ps_spt = ctx.enter_context(tc.tile_pool(name="ps_spt", bufs=2, space="PSUM"))
        ps_o = ctx.enter_context(tc.tile_pool(name="ps_o", bufs=1, space="PSUM"))
        ps_f = ctx.enter_context(tc.tile_pool(name="ps_f", bufs=1, space="PSUM"))
        ps_w = ctx.enter_context(tc.tile_pool(name="ps_w", bufs=1, space="PSUM"))

        # Manual schedule: tile_wait_until floors (in us of virtual time)
        # dictate the per-engine FIFO order the Tile scheduler emits.  The
        # round-3 profile showed the scheduler pushing the score->exp->
        # transpose chains behind bulk projection matmuls (+3.5us critical
        # path) - these floors pin the intended order.
        from contextlib import contextmanager

        @contextmanager
        def at(us):
            # x100 virtual-time scale so floors dominate the scheduler's own
            # DMA-arrival estimates -> the floor order IS the engine order.
            with tc.tile_wait_until(us / 10.0):
                yield

        # ---- loads: 8 bf16 DMAs, two HWDGE engines in parallel ----
        chunk_t = {}
        for i, (lo, hi, eng) in enumerate(CHUNKS):
            with at(0.01 * (i + 1)):
                t = io.tile([128, hi - lo], bf16, tag=f"c{lo}")
                getattr(nc, eng).dma_start(out=t, in_=packA[:, lo:hi])
                chunk_t[lo] = t

        def seg(name, k=0):
            off, width = _seg_off(name, k)
            for lo, hi, _ in CHUNKS:
                if lo <= off and off + width <= hi:
                    return chunk_t[lo][:, off - lo:off - lo + width]
            raise ValueError(f"segment {name}[{k}] crosses chunk boundary")

        # identity (for the normalizing transpose), built on-device:
        # iota(p,f) = p - f, select == 0 from a ones tile.
        ones = io.tile([128, 128], bf16, tag="ones")
        nc.vector.memset(ones, 1.0)
        identb = io.tile([128, 128], bf16, tag="identb")
        nc.gpsimd.affine_select(
            identb, ones, pattern=[[-1, 128]],
            compare_op=mybir.AluOpType.is_equal, fill=0.0,
            base=0, channel_multiplier=1,
        )

        # ---- PE warmup: small bf16 matmuls on memset scratch keep the PE
        #      busy for the whole DMA wait (~3us) so the HAM clock gate is
        #      at 8/8 (2.4 GHz) by the time real matmuls start ----
        warm_sb = io.tile([128, 128], bf16, tag="warm_sb")
        nc.vector.memset(warm_sb, 0.5)
        warm_ps = ps_w.tile([128, 128], f32, tag="w_ps")
        for w in range(24):
            nc.tensor.matmul(
                warm_ps, warm_sb, warm_sb,
                start=(w == 0), stop=(w == 23),
            )

        # ---- pipelined compute ----
        def kproj(wk, x, side, t01, t23, tcp):
            """kT [128ch(2 heads), 256tok] = Wk_side @ x_side.T"""
            kp = ps_mm.tile([128, 256], f32, tag="mm")
            for k in range(4):
                with at(t01 if k < 2 else t23):
                    nc.tensor.matmul(
                        kp, seg(wk, k), seg(x, k),
                        start=(k == 0), stop=(k == 3),
                    )
            with at(tcp):
                ks = work.tile([128, 256], bf16, tag=f"k_sb{side}")
                nc.vector.tensor_copy(ks, kp)
            return ks

        def vproj(wv, x, side, t, tcp):
            """v [128tok x 2 halves, 128ch] projected directly (tokens on
            partitions): v[t] = x_ktile[:, half t].T @ Wv_ktile."""
            vp = ps_mm.tile([128, 2, 128], f32, tag="mm")
            with at(t):
                for th in range(2):
                    for k in range(4):
                        nc.tensor.matmul(
                            vp[:, th, :],
                            seg(x, k)[:, 128 * th:128 * th + 128],
                            seg(wv, k),
                            start=(k == 0), stop=(k == 3),
                        )
            with at(tcp):
                vs = work.tile([128, 2, 128], bf16, tag=f"v_sb{side}")
                nc.scalar.copy(vs, vp)
            return vs

        def scores_softmax(ks, side, ts, texp, tdve):
            """scores (partitions = 64*h + q) -> exp (no max-subtraction:
            |s| < ~2.5 by construction) -> diag(1/z) for the transpose."""
            with at(ts):
                sp = ps_spt.tile([128, 256], f32, tag="spt")
                for h in range(2):
                    hs = slice(64 * h, 64 * h + 64)
                    nc.tensor.matmul(
                        sp[hs, :], q2T[hs, :], ks[hs, :],
                        start=True, stop=True, tile_position=(64 * h, 64 * h),
                    )
            with at(texp):
                p = work.tile([128, 256], bf16, tag=f"p{side}")
                zsum = work.tile([128, 1], f32, tag=f"zsum{side}")
                nc.scalar.activation(
                    p, sp, mybir.ActivationFunctionType.Exp, accum_out=zsum
                )
            with at(tdve):
                zrec = work.tile([128, 1], f32, tag=f"zrec{side}")
                diag = work.tile([128, 128], bf16, tag=f"diag{side}")
                nc.vector.reciprocal(zrec, zsum)
                nc.vector.tensor_scalar_mul(diag, identb, zrec)
            return p, diag

        def ptrans(p, diag, side, t, tcp):
            """transpose p [128(h,q), 256keys] -> [128keys, 2, (h,q)] while
            normalizing: matmul against diag(1/z) instead of the identity."""
            with at(t):
                pt_ps = ps_spt.tile([128, 2, 128], f32, tag="spt")
                for th in range(2):
                    nc.tensor.matmul(
                        pt_ps[:, th, :], p[:, 128 * th:128 * th + 128], diag,
                        start=True, stop=True,
                    )
            with at(tcp):
                pt = work.tile([128, 2, 128], bf16, tag=f"pt_sb{side}")
                nc.vector.tensor_copy(pt, pt_ps)
            return pt

        # q first (its chunk lands first), then the v-side chain, the a-side
        # chain behind it; v-projections fill the PE while exp/1/z run.
        q_ps = ps_mm.tile([128, 64], f32, tag="mm")
        with at(3.10):
            for k in range(4):
                nc.tensor.matmul(
                    q_ps, seg("wq", k), seg("xmm", k),
                    start=(k == 0), stop=(k == 3),
                )
        with at(3.18):
            q2T = work.tile([128, 64], bf16, tag="q2T")
            nc.scalar.mul(q2T, q_ps, SCALE)

        k_v = kproj("wkv", "xv", 0, 3.05, 3.15, 3.20)
        p_v, diag_v = scores_softmax(k_v, 0, 3.25, 3.30, 3.35)
        k_a = kproj("wka", "xa", 1, 3.40, 3.45, 3.42)
        v_v = vproj("wvv", "xv", 0, 3.48, 3.66)
        pt_v = ptrans(p_v, diag_v, 0, 3.50, 3.56)
        p_a, diag_a = scores_softmax(k_a, 1, 3.52, 3.70, 3.74)
        v_a = vproj("wva", "xa", 1, 3.54, 3.92)
        pt_a = ptrans(p_a, diag_a, 1, 3.85, 3.90)

        v_sb = [v_v, v_a]
        pt_sides = [pt_v, pt_a]

        # PV: o[128ch(2 heads), 64q] accumulated per head (col-tiled for h=1)
        o_ps = ps_o.tile([128, 64], f32, tag="o")
        with at(4.00):
            for h in range(2):
                hs = slice(64 * h, 64 * h + 64)
                n = 0
                for side in range(2):
                    for t in range(2):
                        nc.tensor.matmul(
                            o_ps[hs, :],
                            v_sb[side][:, t, hs],
                            pt_sides[side][:, t, 64 * h:64 * h + 64],
                            start=(n == 0), stop=(n == 3),
                            tile_position=(0, 64 * h),
                        )
                        n += 1
        with at(4.05):
            o_sb = work.tile([128, 64], bf16, tag="o_sb")
            nc.vector.tensor_copy(o_sb, o_ps)

        # output projection partial [64q, 512], split in column halves so the
        # first half's store overlaps the second half's matmul + copy.
        f_ps = ps_f.tile([64, 512], f32, tag="f_ps")
        f_sb = work.tile([64, 512], f32, tag="f_sb")
        with at(4.10):
            nc.tensor.matmul(
                f_ps[:, 0:256], o_sb, seg("wproj")[:, 0:256],
                start=True, stop=True,
            )
        with at(4.15):
            nc.vector.tensor_copy(f_sb[:, 0:256], f_ps[:, 0:256])
        with at(4.20):
            nc.sync.dma_start(out=out_d[:, 0:256], in_=f_sb[:, 0:256])
        with at(4.22):
            nc.tensor.matmul(
                f_ps[:, 256:512], o_sb, seg("wproj")[:, 256:512],
                start=True, stop=True,
            )
        with at(4.25):
            nc.scalar.copy(f_sb[:, 256:512], f_ps[:, 256:512])
        with at(4.30):
            nc.scalar.dma_start(out=out_d[:, 256:512], in_=f_sb[:, 256:512])

    nc.finalize()
    return nc


def _ktiles(a):
    """[512, C] K-major -> list of 4 [128, C] k-tiles."""
    return [a[128 * k:128 * k + 128, :] for k in range(4)]


def _shard_inputs(xmm, xa, xv, Wq, Wkv, Wproj):
    """Build the 8 per-core input maps (one packed [128, 5376] bf16 tensor)."""
    in_maps = []
    for core in range(N_CORES):
        b, j = divmod(core, 4)
        r = slice(128 * j, 128 * j + 128)               # head-pair rows in [0,512)
        rv = slice(512 + 128 * j, 512 + 128 * j + 128)  # v rows in Wkv
        wkv = _ktiles(Wkv[r, :512].T)
        xvt = _ktiles(xv[b].T)
        wvv = _ktiles(Wkv[rv, :512].T)
        wva = _ktiles(Wkv[rv, 512:].T)
        wq = _ktiles(Wq[r, :].T)
        xmmt = _ktiles(xmm[b].T)
        wka = _ktiles(Wkv[r, 512:].T)
        xat = _ktiles(xa[b].T)
        pack = np.concatenate(
            [
                wkv[0], wkv[1], xvt[0], xvt[1],      # sync chunk 0..768
                wkv[2], wkv[3], xvt[2], xvt[3],      # sync chunk 768..1536
                wvv[0], wvv[1], wvv[2], wvv[3],      # sync chunk 1536..2048
                wva[0], wva[1], wva[2], wva[3],      # sync chunk 2048..2560
                wq[0], wq[1], wq[2], wq[3],          # scalar chunk 2560..3328
                xmmt[0], xmmt[1], xmmt[2], xmmt[3],
                wka[0], wka[1], xat[0], xat[1],      # scalar chunk 3328..4096
                wka[2], wka[3], xat[2], xat[3],      # scalar chunk 4096..4864
                Wproj[:, 128 * j:128 * j + 128].T,   # scalar chunk 4864..5376
            ],
            axis=1,
        )
        assert pack.shape == (128, PACK_COLS)
        in_maps.append({"packA": np.ascontiguousarray(pack).astype(BF16)})
    return in_maps


def _get_program():
    if "nc" not in _cached:
        _cached["nc"] = _build_program()
    return _cached["nc"]


def _register_ntff_hook():
    """Best-effort: register the axon NTFF profile hook that the container's
    antenv stub doesn't provide, so run_bass_kernel_spmd(trace=True) can
    measure HW exec time. No-op on failure."""
    try:
        import types

        try:
            from antenv.axon_hooks import get_axon_ntff_profile_hook
            if get_axon_ntff_profile_hook() is not None:
                return
        except ImportError:
            pass
        import antenv
        from trn_agent_boot.trn_boot import _ntff_profile_via_ctypes

        hook = _ntff_profile_via_ctypes("/opt/axon/libaxon_pjrt.so")
        mod = types.ModuleType("antenv.axon_hooks")
        mod._hook = hook
        mod.set_axon_ntff_profile_hook = lambda h: setattr(mod, "_hook", h)
        mod.get_axon_ntff_profile_hook = lambda: mod._hook
        sys.modules["antenv.axon_hooks"] = mod
        antenv.axon_hooks = mod

        # artifact upload has no backing store in this container
        from concourse import bass_utils

        bass_utils.upload_artifacts = lambda tmpdir: tmpdir
    except Exception as e:  # pragma: no cover
        print(f"ntff hook registration failed: {e}", file=sys.stderr)


def kernel(xmm, xa, xv, Wq, Wkv, Wproj, bproj, _want_profile=False):
    from concourse.bass_utils import run_bass_kernel_spmd

    if _want_profile:
        _register_ntff_hook()
    nc = _get_program()
    in_maps = _shard_inputs(
        np.asarray(xmm, np.float32), np.asarray(xa, np.float32),
        np.asarray(xv, np.float32), np.asarray(Wq, np.float32),
        np.asarray(Wkv, np.float32), np.asarray(Wproj, np.float32),
    )
    res = run_bass_kernel_spmd(
        nc, in_maps, core_ids=list(range(N_CORES)), trace=_want_profile
    )
    out = np.zeros((B, N_MM, DIM), np.float32)
    for core in range(N_CORES):
        out[core // 4] += res.results[core]["out"]
    out += np.asarray(bproj, np.float32)[None, None, :]
    if _want_profile:
        return out, res
    return out

